# revision 13
# baseline (speedup 1.0000x reference)
"""Trainium2 Bass kernel for the BPR-style soft-label pairwise loss.

Reference math (per graph g of B=16, N=2048 nodes, labels in {0..3}):
  for lvl in 1..3:
    s_lvl   = sum_{i: lab=lvl} sum_{j: lab<lvl} log_sigmoid(x_i - x_j)
    cnt_lvl = n_lvl * n_{<lvl};  mean_lvl = s_lvl/cnt_lvl if cnt>0 else 0
  per_graph = sum(mean_lvl) / max(#valid, 1);  loss = -mean_g(per_graph)

Kernel strategy (data-parallel, 2 graphs per core on 8 cores):
  -log_sigmoid(x_i - x_j) = ln(1 + e^{x_j} * e^{-x_i})
  Host sorts each graph's nodes by label into a class-segmented layout that is
  uniform across graphs (segment size = max class count over all graphs, padded
  slots get e^{x}=0 so ln(1+0)=0 contributes nothing).  On device, for each
  128-row i-tile the WHOLE pairwise tile is produced by a single ScalarE
  activation: Ln(xrep * scale_p + 1) where xrep is the broadcast e^{x_j} row
  and scale_p the per-partition e^{-x_i}.  A one-hot [128,4] matmul contracts
  the i dimension class-resolved into PSUM G[4, j]; DVE segment reduces give
  S[i-class, j-class] sums; the O(1)-sized remainder (counts, divisions,
  averaging) happens on host.  Only pairs with lab_i > lab_j are ever
  evaluated: i-tiles cover classes 1..3, each with j-extent = end of class
  (lab_i - 1)'s segment, so the device does ~3/8 of the dense N^2 work.
"""

import os
import sys

import numpy as np

for _p in ("/root/.axon_site/_ro/trn_rl_repo", "/opt/trn_rl_repo"):
    if os.path.isdir(_p) and _p not in sys.path:
        sys.path.append(_p)

import concourse.bacc as bacc
import concourse.mybir as mybir
import concourse.tile as tile
from concourse.bass_utils import run_bass_kernel_spmd

B, N, NCLS = 16, 2048, 4
N_CORES = 8
GPC = B // N_CORES  # graphs per core
P = 128
AF = mybir.ActivationFunctionType

_BUILD_CACHE = {}


def _layout(scls):
    """Derive the uniform class-segmented layout from per-class segment sizes."""
    s0, s1, s2, s3 = scls
    jstart = [0, s0, s0 + s1, s0 + s1 + s2]  # segment starts for j classes 0..2
    lj = s0 + s1 + s2                        # j layout length (classes 0..2)
    jext = {1: jstart[1], 2: jstart[2], 3: lj}  # j extent per i level
    istart = {1: 0, 2: s1, 3: s1 + s2}       # i layout: classes 1..3
    li_raw = s1 + s2 + s3
    ti = max(0, -(-li_raw // P))             # number of 128-row i tiles
    levels = []
    for t in range(ti):
        lo, hi = P * t, P * (t + 1)
        lv = 0
        for a in (1, 2, 3):
            if scls[a] > 0 and istart[a] < hi and istart[a] + scls[a] > lo:
                lv = a
        levels.append(lv)
    return jstart, lj, jext, istart, li_raw, ti, levels


def _build(scls):
    """Build + compile the SPMD bass program for given segment sizes."""
    jstart, lj, jext, istart, li_raw, ti, levels = _layout(scls)
    f32 = mybir.dt.float32

    nc = bacc.Bacc("TRN2", debug=False, enable_asserts=False, num_devices=N_CORES)
    expxj_d = nc.dram_tensor(
        "expxj", [GPC, max(lj, 1)], mybir.dt.float32r, kind="ExternalInput").ap()
    expnegxi_d = nc.dram_tensor(
        "expnegxi", [GPC, P, max(ti, 1)], f32, kind="ExternalInput").ap()
    onehot_d = nc.dram_tensor(
        "onehot", [GPC, max(ti, 1), P, NCLS], mybir.dt.float32r,
        kind="ExternalInput").ap()
    ones_d = nc.dram_tensor("ones", [1, P], mybir.dt.float32r, kind="ExternalInput").ap()
    r_d = nc.dram_tensor("r", [GPC, 4, 8], f32, kind="ExternalOutput").ap()

    # reduce-output column for each (level a, j class c < a)
    rcol = {}
    for a in (1, 2, 3):
        for c in range(a):
            rcol[(a, c)] = len(rcol)
    assert len(rcol) <= 8

    f32r = mybir.dt.float32r
    with tile.TileContext(nc) as tc:
        with (
            tc.tile_pool(name="sb", bufs=1) as sb,
            tc.tile_pool(name="xrp", bufs=1) as xrp,
            tc.tile_pool(name="vp", bufs=3) as vp,
            tc.tile_pool(name="rp", bufs=2) as rp,
            tc.tile_pool(name="ps", bufs=2, space="PSUM") as ps,
        ):
            # warm-up: force the Ln act-table load before any DMA-dependent op
            warm = sb.tile([1, 1], f32)
            nc.vector.memset(warm[:], 1.0)
            nc.scalar.activation(warm[:], warm[:], AF.Ln, bias=1.0, scale=1.0)

            expnegxi = sb.tile([P, GPC * ti], f32)
            onehot = sb.tile([P, GPC * ti * NCLS], f32r)
            ones = sb.tile([1, P], f32r)
            nc.sync.dma_start(ones[:], ones_d[:])
            # tiny inputs first on the fast queue: e^{x_j} rows + scales
            xjrows = []
            for g in range(GPC):
                xjr = sb.tile([1, lj], f32r, tag=f"xjr{g}", name=f"xjr{g}")
                nc.sync.dma_start(xjr[:], expxj_d[g : g + 1, :])
                xjrows.append(xjr)
            nc.sync.dma_start(
                expnegxi[:].rearrange("p (g t) -> p g t", g=GPC),
                expnegxi_d.rearrange("g p t -> p g t"),
            )
            # bulk one-hot on the gpsimd queue to avoid contending
            nc.gpsimd.dma_start(
                onehot[:].rearrange("p (g t c) -> p g t c", g=GPC, t=ti),
                onehot_d.rearrange("g t p c -> p g t c"),
            )
            # broadcast e^{x_j} rows to all partitions: PE outer product with
            # ones + DVE copy, pipelined per 512-chunk (starts ~1us, no DMA)
            xreps = []
            for g in range(GPC):
                xrep = xrp.tile([P, lj], f32, tag=f"xrep{g}", name=f"xrep{g}")
                for k0 in range(0, lj, 512):
                    k1 = min(k0 + 512, lj)
                    bc = ps.tile([P, 512], f32, tag="bc", bufs=2)
                    nc.tensor.matmul(
                        bc[:, : k1 - k0], ones[:], xjrows[g][:, k0:k1],
                        start=True, stop=True,
                    )
                    nc.vector.tensor_copy(xrep[:, k0:k1], bc[:, : k1 - k0])
                xreps.append(xrep)

            for g in range(GPC):
                xrep = xreps[g]
                r_sb = rp.tile([4, 8], f32, tag="r")
                nc.vector.memset(r_sb[:], 0.0)

                # graph 0 ascending (earliest data need is smallest chunk),
                # last graph descending (kernel tail ends on the small level)
                order = (1, 2, 3) if g < GPC - 1 else (3, 2, 1)
                for a in order:
                    tiles = [t for t in range(ti) if levels[t] == a]
                    ext = jext[a]
                    if not tiles or ext == 0:
                        continue
                    g_ps = ps.tile([4, lj], f32, tag="g", bufs=1)
                    for idx, t in enumerate(tiles):
                        col = g * ti + t
                        v = vp.tile([P, ext], f32r, tag="v")
                        nc.scalar.activation(
                            v[:], xrep[:, :ext], AF.Ln,
                            bias=1.0, scale=expnegxi[:, col : col + 1],
                        )
                        for k0 in range(0, ext, 512):
                            k1 = min(k0 + 512, ext)
                            nc.tensor.matmul(
                                g_ps[:, k0:k1],
                                onehot[:, col * NCLS : (col + 1) * NCLS],
                                v[:, k0:k1],
                                start=(idx == 0),
                                stop=(idx == len(tiles) - 1),
                            )
                    for c in range(a):
                        c0, c1 = jstart[c], jstart[c] + scls[c]
                        if c1 <= c0:
                            continue
                        cc = rcol[(a, c)]
                        nc.vector.reduce_sum(
                            r_sb[:, cc : cc + 1], g_ps[:, c0:c1],
                            axis=mybir.AxisListType.X,
                        )
                nc.sync.dma_start(r_d[g], r_sb[:, :])
    nc.compile()
    return nc, rcol


def _prepare_core(logits, labels, scls):
    """Host-side layout prep for one core's GPC graphs."""
    jstart, lj, jext, istart, li_raw, ti, levels = _layout(scls)
    expxj = np.zeros((GPC, max(lj, 1)), np.float32)
    expnegxi = np.zeros((GPC, P, max(ti, 1)), np.float32)
    onehot = np.zeros((GPC, max(ti, 1), P, NCLS), np.float32)
    for g in range(GPC):
        x = logits[g].astype(np.float64)
        lab = labels[g]
        for c in (0, 1, 2):
            xc = x[lab == c]
            expxj[g, jstart[c] : jstart[c] + xc.size] = np.exp(xc)
        ivals = np.zeros(P * max(ti, 1), np.float64)
        ioh = np.zeros((P * max(ti, 1), NCLS), np.float32)
        for a in (1, 2, 3):
            xa = x[lab == a]
            i0 = istart[a]
            ivals[i0 : i0 + xa.size] = np.exp(-xa)
            ioh[i0 : i0 + xa.size, a] = 1.0
        expnegxi[g] = ivals.reshape(max(ti, 1), P).T.astype(np.float32)
        onehot[g] = ioh.reshape(max(ti, 1), P, NCLS)
    return {"expxj": expxj, "expnegxi": expnegxi, "onehot": onehot,
            "ones": np.ones((1, P), np.float32)}


def _assemble(r_all, counts, rcol):
    """Host-side O(1) final math from device segment sums. r_all: [B, 4, 8]."""
    per_graph = np.zeros(B, np.float64)
    for g in range(B):
        n = counts[g]
        means = []
        valids = []
        for lvl in (1, 2, 3):
            s_dev = 0.0
            for a in range(lvl, 4):
                for c in range(lvl):
                    if (a, c) in rcol:
                        s_dev += float(r_all[g, lvl, rcol[(a, c)]])
            s_ref = -s_dev
            cnt = float(n[lvl]) * float(n[:lvl].sum())
            valid = cnt > 0
            means.append(s_ref / max(cnt, 1.0) if valid else 0.0)
            valids.append(1.0 if valid else 0.0)
        per_graph[g] = sum(means) / max(sum(valids), 1.0)
    return np.float32(-per_graph.mean())


def kernel(logits, labels):
    logits = np.ascontiguousarray(np.asarray(logits, np.float32))
    labels = np.ascontiguousarray(np.asarray(labels, np.int32))
    assert logits.shape == (B, N) and labels.shape == (B, N)

    counts = np.stack([(labels == c).sum(1) for c in range(NCLS)], axis=1)  # [B,4]
    # fp32r matmuls need even free-dim counts -> round segment sizes to even
    scls = tuple(int(counts[:, c].max() + 1) // 2 * 2 for c in range(NCLS))

    if scls not in _BUILD_CACHE:
        _BUILD_CACHE[scls] = _build(scls)
    nc, rcol = _BUILD_CACHE[scls]

    in_maps = [
        _prepare_core(logits[c * GPC : (c + 1) * GPC],
                      labels[c * GPC : (c + 1) * GPC], scls)
        for c in range(N_CORES)
    ]
    res = run_bass_kernel_spmd(nc, in_maps, list(range(N_CORES)))
    r_all = np.concatenate([res.results[c]["r"] for c in range(N_CORES)], axis=0)
    return _assemble(r_all, counts, rcol)


if __name__ == "__main__":
    rng = np.random.default_rng(0)
    lg = rng.normal(size=(B, N)).astype(np.float32)
    lb = rng.integers(0, NCLS, size=(B, N)).astype(np.int32)
    print(kernel(lg, lb))


# revision 16
# speedup vs baseline: 1.0062x; 1.0062x over previous
"""Trainium2 Bass kernel for the BPR-style soft-label pairwise loss.

Reference math (per graph g of B=16, N=2048 nodes, labels in {0..3}):
  for lvl in 1..3:
    s_lvl   = sum_{i: lab=lvl} sum_{j: lab<lvl} log_sigmoid(x_i - x_j)
    cnt_lvl = n_lvl * n_{<lvl};  mean_lvl = s_lvl/cnt_lvl if cnt>0 else 0
  per_graph = sum(mean_lvl) / max(#valid, 1);  loss = -mean_g(per_graph)

Kernel strategy (data-parallel, 2 graphs per core on 8 cores):
  -log_sigmoid(x_i - x_j) = ln(1 + e^{x_j} * e^{-x_i})
  Host sorts each graph's nodes by label into a class-segmented layout that is
  uniform across graphs (segment size = max class count over all graphs, padded
  slots get e^{x}=0 so ln(1+0)=0 contributes nothing).  On device, for each
  128-row i-tile the WHOLE pairwise tile is produced by a single ScalarE
  activation: Ln(xrep * scale_p + 1) where xrep is the broadcast e^{x_j} row
  and scale_p the per-partition e^{-x_i}.  A one-hot [128,4] matmul contracts
  the i dimension class-resolved into PSUM G[4, j]; DVE segment reduces give
  S[i-class, j-class] sums; the O(1)-sized remainder (counts, divisions,
  averaging) happens on host.  Only pairs with lab_i > lab_j are ever
  evaluated: i-tiles cover classes 1..3, each with j-extent = end of class
  (lab_i - 1)'s segment, so the device does ~3/8 of the dense N^2 work.
"""

import os
import sys

import numpy as np

for _p in ("/root/.axon_site/_ro/trn_rl_repo", "/opt/trn_rl_repo"):
    if os.path.isdir(_p) and _p not in sys.path:
        sys.path.append(_p)

import concourse.bacc as bacc
import concourse.mybir as mybir
import concourse.tile as tile
from concourse.bass_utils import run_bass_kernel_spmd

B, N, NCLS = 16, 2048, 4
N_CORES = 8
GPC = B // N_CORES  # graphs per core
P = 128
AF = mybir.ActivationFunctionType

_BUILD_CACHE = {}


def _layout(scls):
    """Derive the uniform class-segmented layout from per-class segment sizes."""
    s0, s1, s2, s3 = scls
    jstart = [0, s0, s0 + s1, s0 + s1 + s2]  # segment starts for j classes 0..2
    lj = s0 + s1 + s2                        # j layout length (classes 0..2)
    jext = {1: jstart[1], 2: jstart[2], 3: lj}  # j extent per i level
    istart = {1: 0, 2: s1, 3: s1 + s2}       # i layout: classes 1..3
    li_raw = s1 + s2 + s3
    ti = max(0, -(-li_raw // P))             # number of 128-row i tiles
    levels = []
    for t in range(ti):
        lo, hi = P * t, P * (t + 1)
        lv = 0
        for a in (1, 2, 3):
            if scls[a] > 0 and istart[a] < hi and istart[a] + scls[a] > lo:
                lv = a
        levels.append(lv)
    return jstart, lj, jext, istart, li_raw, ti, levels


def _build(scls):
    """Build + compile the SPMD bass program for given segment sizes."""
    jstart, lj, jext, istart, li_raw, ti, levels = _layout(scls)
    f32 = mybir.dt.float32

    nc = bacc.Bacc("TRN2", debug=False, enable_asserts=False, num_devices=N_CORES)
    expxj_d = nc.dram_tensor(
        "expxj", [GPC, max(lj, 1)], f32, kind="ExternalInput").ap()
    expnegxi_d = nc.dram_tensor(
        "expnegxi", [GPC, P, max(ti, 1)], f32, kind="ExternalInput").ap()
    onehot_d = nc.dram_tensor(
        "onehot", [GPC, max(ti, 1), P, NCLS], mybir.dt.float32r,
        kind="ExternalInput").ap()
    r_d = nc.dram_tensor("r", [GPC, 4, 8], f32, kind="ExternalOutput").ap()

    # reduce-output column for each (level a, j class c < a)
    rcol = {}
    for a in (1, 2, 3):
        for c in range(a):
            rcol[(a, c)] = len(rcol)
    assert len(rcol) <= 8

    f32r = mybir.dt.float32r
    with tile.TileContext(nc) as tc:
        with (
            tc.tile_pool(name="sb", bufs=1) as sb,
            tc.tile_pool(name="xrp", bufs=1) as xrp,
            tc.tile_pool(name="vp", bufs=3) as vp,
            tc.tile_pool(name="rp", bufs=2) as rp,
            tc.tile_pool(name="ps", bufs=2, space="PSUM") as ps,
        ):
            # warm-up: force the Ln act-table load before any DMA-dependent op
            warm = sb.tile([1, 1], f32)
            nc.vector.memset(warm[:], 1.0)
            nc.scalar.activation(warm[:], warm[:], AF.Ln, bias=1.0, scale=1.0)

            expnegxi = sb.tile([P, GPC * ti], f32)
            onehot = sb.tile([P, GPC * ti * NCLS], f32r)
            # small inputs on the gpsimd ring: scales first (first-ACT dep),
            # then the bulk one-hot (first-matmul dep)
            nc.gpsimd.dma_start(
                expnegxi[:].rearrange("p (g t) -> p g t", g=GPC),
                expnegxi_d.rearrange("g p t -> p g t"),
            )
            nc.gpsimd.dma_start(
                onehot[:].rearrange("p (g t c) -> p g t c", g=GPC, t=ti),
                onehot_d.rearrange("g t p c -> p g t c"),
            )
            # broadcast DMAs on the HWDGE ring: graph-0 split at level
            # boundaries so level-1 ACTs start as soon as chunk 0 lands
            xreps = []
            bnds = sorted({jext[a] for a in (1, 2, 3) if jext[a] > 0})
            for g in range(GPC):
                xreps.append(
                    xrp.tile([P, lj], f32, tag=f"xrep{g}", name=f"xrep{g}"))
            prev = 0
            for b in bnds:
                nc.sync.dma_start(
                    xreps[0][:, prev:b],
                    expxj_d[0:1, prev:b].broadcast_to([P, b - prev]),
                )
                prev = b
            for g in range(1, GPC):
                nc.sync.dma_start(
                    xreps[g][:], expxj_d[g : g + 1, :].broadcast_to([P, lj]))

            for g in range(GPC):
                xrep = xreps[g]
                r_sb = rp.tile([4, 8], f32, tag="r")
                nc.vector.memset(r_sb[:], 0.0)

                # graph 0 ascending (earliest data need is smallest chunk),
                # last graph descending (kernel tail ends on the small level)
                order = (1, 2, 3) if g < GPC - 1 else (3, 2, 1)
                for a in order:
                    tiles = [t for t in range(ti) if levels[t] == a]
                    ext = jext[a]
                    if not tiles or ext == 0:
                        continue
                    g_ps = ps.tile([4, lj], f32, tag="g", bufs=2)
                    for idx, t in enumerate(tiles):
                        col = g * ti + t
                        v = vp.tile([P, ext], f32r, tag="v")
                        nc.scalar.activation(
                            v[:], xrep[:, :ext], AF.Ln,
                            bias=1.0, scale=expnegxi[:, col : col + 1],
                        )
                        for k0 in range(0, ext, 512):
                            k1 = min(k0 + 512, ext)
                            nc.tensor.matmul(
                                g_ps[:, k0:k1],
                                onehot[:, col * NCLS : (col + 1) * NCLS],
                                v[:, k0:k1],
                                start=(idx == 0),
                                stop=(idx == len(tiles) - 1),
                            )
                    for c in range(a):
                        c0, c1 = jstart[c], jstart[c] + scls[c]
                        if c1 <= c0:
                            continue
                        cc = rcol[(a, c)]
                        nc.vector.reduce_sum(
                            r_sb[:, cc : cc + 1], g_ps[:, c0:c1],
                            axis=mybir.AxisListType.X,
                        )
                nc.sync.dma_start(r_d[g], r_sb[:, :])
    nc.compile()
    return nc, rcol


def _prepare_core(logits, labels, scls):
    """Host-side layout prep for one core's GPC graphs."""
    jstart, lj, jext, istart, li_raw, ti, levels = _layout(scls)
    expxj = np.zeros((GPC, max(lj, 1)), np.float32)
    expnegxi = np.zeros((GPC, P, max(ti, 1)), np.float32)
    onehot = np.zeros((GPC, max(ti, 1), P, NCLS), np.float32)
    for g in range(GPC):
        x = logits[g].astype(np.float64)
        lab = labels[g]
        for c in (0, 1, 2):
            xc = x[lab == c]
            expxj[g, jstart[c] : jstart[c] + xc.size] = np.exp(xc)
        ivals = np.zeros(P * max(ti, 1), np.float64)
        ioh = np.zeros((P * max(ti, 1), NCLS), np.float32)
        for a in (1, 2, 3):
            xa = x[lab == a]
            i0 = istart[a]
            ivals[i0 : i0 + xa.size] = np.exp(-xa)
            ioh[i0 : i0 + xa.size, a] = 1.0
        expnegxi[g] = ivals.reshape(max(ti, 1), P).T.astype(np.float32)
        onehot[g] = ioh.reshape(max(ti, 1), P, NCLS)
    return {"expxj": expxj, "expnegxi": expnegxi, "onehot": onehot}


def _assemble(r_all, counts, rcol):
    """Host-side O(1) final math from device segment sums. r_all: [B, 4, 8]."""
    per_graph = np.zeros(B, np.float64)
    for g in range(B):
        n = counts[g]
        means = []
        valids = []
        for lvl in (1, 2, 3):
            s_dev = 0.0
            for a in range(lvl, 4):
                for c in range(lvl):
                    if (a, c) in rcol:
                        s_dev += float(r_all[g, lvl, rcol[(a, c)]])
            s_ref = -s_dev
            cnt = float(n[lvl]) * float(n[:lvl].sum())
            valid = cnt > 0
            means.append(s_ref / max(cnt, 1.0) if valid else 0.0)
            valids.append(1.0 if valid else 0.0)
        per_graph[g] = sum(means) / max(sum(valids), 1.0)
    return np.float32(-per_graph.mean())


def kernel(logits, labels):
    logits = np.ascontiguousarray(np.asarray(logits, np.float32))
    labels = np.ascontiguousarray(np.asarray(labels, np.int32))
    assert logits.shape == (B, N) and labels.shape == (B, N)

    counts = np.stack([(labels == c).sum(1) for c in range(NCLS)], axis=1)  # [B,4]
    # fp32r matmuls need even free-dim counts -> round segment sizes to even
    scls = tuple(int(counts[:, c].max() + 1) // 2 * 2 for c in range(NCLS))

    if scls not in _BUILD_CACHE:
        _BUILD_CACHE[scls] = _build(scls)
    nc, rcol = _BUILD_CACHE[scls]

    in_maps = [
        _prepare_core(logits[c * GPC : (c + 1) * GPC],
                      labels[c * GPC : (c + 1) * GPC], scls)
        for c in range(N_CORES)
    ]
    res = run_bass_kernel_spmd(nc, in_maps, list(range(N_CORES)))
    r_all = np.concatenate([res.results[c]["r"] for c in range(N_CORES)], axis=0)
    return _assemble(r_all, counts, rcol)


if __name__ == "__main__":
    rng = np.random.default_rng(0)
    lg = rng.normal(size=(B, N)).astype(np.float32)
    lb = rng.integers(0, NCLS, size=(B, N)).astype(np.int32)
    print(kernel(lg, lb))


# revision 19
# speedup vs baseline: 1.0947x; 1.0880x over previous
"""Trainium2 Bass kernel for the BPR-style soft-label pairwise loss.

Reference math (per graph g of B=16, N=2048 nodes, labels in {0..3}):
  for lvl in 1..3:
    s_lvl   = sum_{i: lab=lvl} sum_{j: lab<lvl} log_sigmoid(x_i - x_j)
    cnt_lvl = n_lvl * n_{<lvl};  mean_lvl = s_lvl/cnt_lvl if cnt>0 else 0
  per_graph = sum(mean_lvl) / max(#valid, 1);  loss = -mean_g(per_graph)

Kernel strategy (data-parallel, 2 graphs per core on 8 cores):
  -log_sigmoid(x_i - x_j) = ln(1 + e^{x_j} * e^{-x_i})
  Host sorts each graph's nodes by label into a class-segmented layout that is
  uniform across graphs (segment size = max class count over all graphs, padded
  slots get e^{x}=0 so ln(1+0)=0 contributes nothing).  On device, for each
  128-row i-tile the WHOLE pairwise tile is produced by a single ScalarE
  activation: Ln(xrep * scale_p + 1) where xrep is the broadcast e^{x_j} row
  and scale_p the per-partition e^{-x_i}.  A one-hot [128,4] matmul contracts
  the i dimension class-resolved into PSUM G[4, j]; DVE segment reduces give
  S[i-class, j-class] sums; the O(1)-sized remainder (counts, divisions,
  averaging) happens on host.  Only pairs with lab_i > lab_j are ever
  evaluated: i-tiles cover classes 1..3, each with j-extent = end of class
  (lab_i - 1)'s segment, so the device does ~3/8 of the dense N^2 work.
"""

import os
import sys

import numpy as np

for _p in ("/root/.axon_site/_ro/trn_rl_repo", "/opt/trn_rl_repo"):
    if os.path.isdir(_p) and _p not in sys.path:
        sys.path.append(_p)

import concourse.bacc as bacc
import concourse.mybir as mybir
import concourse.tile as tile
from concourse.bass_utils import run_bass_kernel_spmd

B, N, NCLS = 16, 2048, 4
N_CORES = 8
GPC = B // N_CORES  # graphs per core
P = 128
AF = mybir.ActivationFunctionType

_BUILD_CACHE = {}


def _layout(scls):
    """Derive the uniform class-segmented layout from per-class segment sizes."""
    s0, s1, s2, s3 = scls
    jstart = [0, s0, s0 + s1, s0 + s1 + s2]  # segment starts for j classes 0..2
    lj = s0 + s1 + s2                        # j layout length (classes 0..2)
    jext = {1: jstart[1], 2: jstart[2], 3: lj}  # j extent per i level
    istart = {1: 0, 2: s1, 3: s1 + s2}       # i layout: classes 1..3
    li_raw = s1 + s2 + s3
    ti = max(0, -(-li_raw // P))             # number of 128-row i tiles
    levels = []
    for t in range(ti):
        lo, hi = P * t, P * (t + 1)
        lv = 0
        for a in (1, 2, 3):
            if scls[a] > 0 and istart[a] < hi and istart[a] + scls[a] > lo:
                lv = a
        levels.append(lv)
    return jstart, lj, jext, istart, li_raw, ti, levels


def _build(scls):
    """Build + compile the SPMD bass program for given segment sizes."""
    jstart, lj, jext, istart, li_raw, ti, levels = _layout(scls)
    f32 = mybir.dt.float32

    nc = bacc.Bacc("TRN2", debug=False, enable_asserts=False, num_devices=N_CORES)
    expxj_d = nc.dram_tensor(
        "expxj", [GPC, max(lj, 1)], f32, kind="ExternalInput").ap()
    expnegxi_d = nc.dram_tensor(
        "expnegxi", [P, GPC * max(ti, 1)], f32, kind="ExternalInput").ap()
    onehot_d = nc.dram_tensor(
        "onehot", [P, GPC * max(ti, 1) * NCLS], mybir.dt.float32r,
        kind="ExternalInput").ap()
    r_d = nc.dram_tensor("r", [GPC, 4, 8], f32, kind="ExternalOutput").ap()

    # reduce-output column for each (level a, j class c < a)
    rcol = {}
    for a in (1, 2, 3):
        for c in range(a):
            rcol[(a, c)] = len(rcol)
    assert len(rcol) <= 8

    f32r = mybir.dt.float32r
    with tile.TileContext(nc) as tc:
        with (
            tc.tile_pool(name="sb", bufs=1) as sb,
            tc.tile_pool(name="xrp", bufs=1) as xrp,
            tc.tile_pool(name="vp", bufs=3) as vp,
            tc.tile_pool(name="rp", bufs=2) as rp,
            tc.tile_pool(name="ps", bufs=2, space="PSUM") as ps,
        ):
            # warm-up: force the Ln act-table load before any DMA-dependent op
            warm = sb.tile([1, 1], f32)
            nc.vector.memset(warm[:], 1.0)
            nc.scalar.activation(warm[:], warm[:], AF.Ln, bias=1.0, scale=1.0)

            expnegxi = sb.tile([P, GPC * ti], f32)
            onehot = sb.tile([P, GPC * ti * NCLS], f32r)
            # small inputs first (host supplies them in SBUF layout so the
            # DMAs are plain copies with trivial descriptors)
            nc.sync.dma_start(expnegxi[:], expnegxi_d[:])
            nc.sync.dma_start(onehot[:], onehot_d[:])
            # broadcast DMAs on the HWDGE ring: graph-0 split at level
            # boundaries so level-1 ACTs start as soon as chunk 0 lands
            xreps = []
            bnds = sorted({jext[a] for a in (1, 2, 3) if jext[a] > 0})
            for g in range(GPC):
                xreps.append(
                    xrp.tile([P, lj], f32, tag=f"xrep{g}", name=f"xrep{g}"))
            prev = 0
            for b in bnds:
                nc.sync.dma_start(
                    xreps[0][:, prev:b],
                    expxj_d[0:1, prev:b].broadcast_to([P, b - prev]),
                )
                prev = b
            for g in range(1, GPC):
                nc.sync.dma_start(
                    xreps[g][:], expxj_d[g : g + 1, :].broadcast_to([P, lj]))

            for g in range(GPC):
                xrep = xreps[g]
                r_sb = rp.tile([4, 8], f32, tag="r")
                nc.vector.memset(r_sb[:], 0.0)

                # graph 0 ascending (earliest data need is smallest chunk),
                # last graph descending (kernel tail ends on the small level)
                order = (1, 2, 3) if g < GPC - 1 else (3, 2, 1)
                for a in order:
                    tiles = [t for t in range(ti) if levels[t] == a]
                    ext = jext[a]
                    if not tiles or ext == 0:
                        continue
                    g_ps = ps.tile([4, lj], f32, tag="g", bufs=2)
                    for idx, t in enumerate(tiles):
                        col = g * ti + t
                        v = vp.tile([P, ext], f32r, tag="v")
                        nc.scalar.activation(
                            v[:], xrep[:, :ext], AF.Ln,
                            bias=1.0, scale=expnegxi[:, col : col + 1],
                        )
                        for k0 in range(0, ext, 512):
                            k1 = min(k0 + 512, ext)
                            nc.tensor.matmul(
                                g_ps[:, k0:k1],
                                onehot[:, col * NCLS : (col + 1) * NCLS],
                                v[:, k0:k1],
                                start=(idx == 0),
                                stop=(idx == len(tiles) - 1),
                            )
                    for c in range(a):
                        c0, c1 = jstart[c], jstart[c] + scls[c]
                        if c1 <= c0:
                            continue
                        cc = rcol[(a, c)]
                        nc.vector.reduce_sum(
                            r_sb[:, cc : cc + 1], g_ps[:, c0:c1],
                            axis=mybir.AxisListType.X,
                        )
                nc.sync.dma_start(r_d[g], r_sb[:, :])
    nc.compile()
    return nc, rcol


def _prepare_core(logits, labels, scls):
    """Host-side layout prep for one core's GPC graphs."""
    jstart, lj, jext, istart, li_raw, ti, levels = _layout(scls)
    expxj = np.zeros((GPC, max(lj, 1)), np.float32)
    expnegxi = np.zeros((GPC, P, max(ti, 1)), np.float32)
    onehot = np.zeros((GPC, max(ti, 1), P, NCLS), np.float32)
    for g in range(GPC):
        x = logits[g].astype(np.float64)
        lab = labels[g]
        for c in (0, 1, 2):
            xc = x[lab == c]
            expxj[g, jstart[c] : jstart[c] + xc.size] = np.exp(xc)
        ivals = np.zeros(P * max(ti, 1), np.float64)
        ioh = np.zeros((P * max(ti, 1), NCLS), np.float32)
        for a in (1, 2, 3):
            xa = x[lab == a]
            i0 = istart[a]
            ivals[i0 : i0 + xa.size] = np.exp(-xa)
            ioh[i0 : i0 + xa.size, a] = 1.0
        expnegxi[g] = ivals.reshape(max(ti, 1), P).T.astype(np.float32)
        onehot[g] = ioh.reshape(max(ti, 1), P, NCLS)
    # rearrange to the exact SBUF layout: [P, g*t] and [P, g*t*c]
    expnegxi_sb = np.ascontiguousarray(
        expnegxi.transpose(1, 0, 2).reshape(P, GPC * max(ti, 1)))
    onehot_sb = np.ascontiguousarray(
        onehot.transpose(2, 0, 1, 3).reshape(P, GPC * max(ti, 1) * NCLS))
    return {"expxj": expxj, "expnegxi": expnegxi_sb, "onehot": onehot_sb}


def _assemble(r_all, counts, rcol):
    """Host-side O(1) final math from device segment sums. r_all: [B, 4, 8]."""
    per_graph = np.zeros(B, np.float64)
    for g in range(B):
        n = counts[g]
        means = []
        valids = []
        for lvl in (1, 2, 3):
            s_dev = 0.0
            for a in range(lvl, 4):
                for c in range(lvl):
                    if (a, c) in rcol:
                        s_dev += float(r_all[g, lvl, rcol[(a, c)]])
            s_ref = -s_dev
            cnt = float(n[lvl]) * float(n[:lvl].sum())
            valid = cnt > 0
            means.append(s_ref / max(cnt, 1.0) if valid else 0.0)
            valids.append(1.0 if valid else 0.0)
        per_graph[g] = sum(means) / max(sum(valids), 1.0)
    return np.float32(-per_graph.mean())


def kernel(logits, labels):
    logits = np.ascontiguousarray(np.asarray(logits, np.float32))
    labels = np.ascontiguousarray(np.asarray(labels, np.int32))
    assert logits.shape == (B, N) and labels.shape == (B, N)

    counts = np.stack([(labels == c).sum(1) for c in range(NCLS)], axis=1)  # [B,4]
    # fp32r matmuls need even free-dim counts -> round segment sizes to even
    scls = tuple(int(counts[:, c].max() + 1) // 2 * 2 for c in range(NCLS))

    if scls not in _BUILD_CACHE:
        _BUILD_CACHE[scls] = _build(scls)
    nc, rcol = _BUILD_CACHE[scls]

    in_maps = [
        _prepare_core(logits[c * GPC : (c + 1) * GPC],
                      labels[c * GPC : (c + 1) * GPC], scls)
        for c in range(N_CORES)
    ]
    res = run_bass_kernel_spmd(nc, in_maps, list(range(N_CORES)))
    r_all = np.concatenate([res.results[c]["r"] for c in range(N_CORES)], axis=0)
    return _assemble(r_all, counts, rcol)


if __name__ == "__main__":
    rng = np.random.default_rng(0)
    lg = rng.normal(size=(B, N)).astype(np.float32)
    lb = rng.integers(0, NCLS, size=(B, N)).astype(np.int32)
    print(kernel(lg, lb))


# revision 23
# speedup vs baseline: 1.1143x; 1.0179x over previous
"""Trainium2 Bass kernel for the BPR-style soft-label pairwise loss.

Reference math (per graph g of B=16, N=2048 nodes, labels in {0..3}):
  for lvl in 1..3:
    s_lvl   = sum_{i: lab=lvl} sum_{j: lab<lvl} log_sigmoid(x_i - x_j)
    cnt_lvl = n_lvl * n_{<lvl};  mean_lvl = s_lvl/cnt_lvl if cnt>0 else 0
  per_graph = sum(mean_lvl) / max(#valid, 1);  loss = -mean_g(per_graph)

Kernel strategy (data-parallel, 2 graphs per core on 8 cores):
  -log_sigmoid(x_i - x_j) = ln(1 + e^{x_j} * e^{-x_i})
  The host sorts each graph's nodes by label into a class-segmented layout
  that is uniform across graphs (segment size = max class count over all
  graphs rounded to even; padded slots carry e^{x}=0 so ln(1+0)=0 contributes
  nothing).  Only pairs with lab_i > lab_j are ever evaluated: i-tiles cover
  classes 1..3, each with j-extent = end of class (lab_i - 1)'s segment, so
  the device does ~3/8 of the dense N^2 transcendental work.

  Per 128-row i-tile the DVE forms t = xrep * e^{-x_i} (bf16 tensor_scalar,
  4x mode) into a grouped buffer; one ScalarE Ln instruction (bias=1) covers
  a whole group of tiles (ACT cost is per-column, so fewer instructions means
  less fixed overhead).  A one-hot [128,4] bf16 matmul contracts the i
  dimension class-resolved into PSUM, with each 512-wide j-chunk accumulating
  into its own 4-row partition band of a single PSUM bank; one DVE copy
  drains a level's whole G to SBUF, a DMA exports it, and the O(N) segment
  sums plus O(1) count/divide/average logic run on host in float64.
"""

import os
import sys

import numpy as np

for _p in ("/root/.axon_site/_ro/trn_rl_repo", "/opt/trn_rl_repo"):
    if os.path.isdir(_p) and _p not in sys.path:
        sys.path.append(_p)

import concourse.bacc as bacc
import concourse.mybir as mybir
import concourse.tile as tile
from concourse.bass_utils import run_bass_kernel_spmd

B, N, NCLS = 16, 2048, 4
N_CORES = 8
GPC = B // N_CORES  # graphs per core
P = 128
CH = 512           # PSUM bank chunk (f32 columns)
AF = mybir.ActivationFunctionType

_BUILD_CACHE = {}


def _layout(scls):
    """Derive the uniform class-segmented layout from per-class segment sizes."""
    s0, s1, s2, s3 = scls
    jstart = [0, s0, s0 + s1, s0 + s1 + s2]  # segment starts for j classes 0..2
    lj = s0 + s1 + s2                        # j layout length (classes 0..2)
    jext = {1: jstart[1], 2: jstart[2], 3: lj}  # j extent per i level
    istart = {1: 0, 2: s1, 3: s1 + s2}       # i layout: classes 1..3
    li_raw = s1 + s2 + s3
    ti = max(0, -(-li_raw // P))             # number of 128-row i tiles
    levels = []
    for t in range(ti):
        lo, hi = P * t, P * (t + 1)
        lv = 0
        for a in (1, 2, 3):
            if scls[a] > 0 and istart[a] < hi and istart[a] + scls[a] > lo:
                lv = a
        levels.append(lv)
    return jstart, lj, jext, istart, li_raw, ti, levels


def _groups(tiles):
    """Split a level's tile list into ACT merge groups of 2-3 tiles."""
    out = []
    i = 0
    while i < len(tiles):
        n = 3 if len(tiles) - i == 3 else 2
        out.append(tiles[i : i + n])
        i += n
    return out


def _build(scls):
    """Build + compile the SPMD bass program for given segment sizes."""
    jstart, lj, jext, istart, li_raw, ti, levels = _layout(scls)
    f32 = mybir.dt.float32
    bf16 = mybir.dt.bfloat16
    max_chunks = max(1, -(-lj // CH))

    nc = bacc.Bacc("TRN2", debug=False, enable_asserts=False, num_devices=N_CORES)
    expxj_d = nc.dram_tensor(
        "expxj", [GPC, max(lj, 1)], bf16, kind="ExternalInput").ap()
    expnegxi_d = nc.dram_tensor(
        "expnegxi", [P, GPC * max(ti, 1)], f32, kind="ExternalInput").ap()
    onehot_d = nc.dram_tensor(
        "onehot", [P, GPC * max(ti, 1) * NCLS], bf16, kind="ExternalInput").ap()
    # packed G export: chunk k lives in piece p=k//3, band b=k%3 (PSUM base
    # partitions must be 0/32/64), rows [32b, 32b+4)
    grows = 68
    npieces = -(-max_chunks // 3)
    gout_d = nc.dram_tensor(
        "gout", [GPC, 3, npieces, grows, CH], f32, kind="ExternalOutput").ap()

    with tile.TileContext(nc) as tc:
        with (
            tc.tile_pool(name="sb", bufs=1) as sb,
            tc.tile_pool(name="xrp", bufs=1) as xrp,
            tc.tile_pool(name="tp", bufs=2) as tp,
            tc.tile_pool(name="vp", bufs=2) as vp,
            tc.tile_pool(name="gsp", bufs=2) as gsp,
            tc.tile_pool(name="ps", bufs=2, space="PSUM") as ps,
        ):
            # warm-up: force the Ln act-table load before any DMA-dependent op
            warm = sb.tile([1, 1], f32)
            nc.vector.memset(warm[:], 1.0)
            nc.scalar.activation(warm[:], warm[:], AF.Ln, bias=1.0, scale=1.0)

            expnegxi = sb.tile([P, GPC * ti], f32)
            onehot = sb.tile([P, GPC * ti * NCLS], bf16)
            # small inputs first (host supplies them in SBUF layout so the
            # DMAs are plain copies with trivial descriptors)
            nc.sync.dma_start(expnegxi[:], expnegxi_d[:])
            nc.sync.dma_start(onehot[:], onehot_d[:])
            # broadcast DMAs: graph-0 split at level boundaries so its
            # level-1 work starts as soon as chunk 0 lands
            xreps = []
            bnds = sorted({jext[a] for a in (1, 2, 3) if jext[a] > 0})
            for g in range(GPC):
                xreps.append(
                    xrp.tile([P, lj], bf16, tag=f"xrep{g}", name=f"xrep{g}"))
            prev = 0
            for b in bnds:
                nc.sync.dma_start(
                    xreps[0][:, prev:b],
                    expxj_d[0:1, prev:b].broadcast_to([P, b - prev]),
                )
                prev = b
            for g in range(1, GPC):
                nc.sync.dma_start(
                    xreps[g][:], expxj_d[g : g + 1, :].broadcast_to([P, lj]))

            # max ACT merge-group width (columns) for t/v buffer sizing
            gw_max = 1
            for a in (1, 2, 3):
                tl = [t for t in range(ti) if levels[t] == a]
                for grp in _groups(tl):
                    gw_max = max(gw_max, len(grp) * jext[a])

            for g in range(GPC):
                xrep = xreps[g]
                order = (1, 2, 3) if g < GPC - 1 else (3, 2, 1)
                for a in order:
                    tiles = [t for t in range(ti) if levels[t] == a]
                    ext = jext[a]
                    if not tiles or ext == 0:
                        continue
                    nch = -(-ext // CH)
                    lv_pieces = -(-nch // 3)
                    g_pss = [
                        ps.tile([grows, CH], f32, tag="g", name=f"g_ps{p}")
                        for p in range(lv_pieces)
                    ]
                    done = 0
                    for grp in _groups(tiles):
                        gw = len(grp) * ext
                        tbuf = tp.tile([P, gw_max], bf16, tag="t", name="tbuf")
                        vbuf = vp.tile([P, gw_max], bf16, tag="v", name="vbuf")
                        for q, t in enumerate(grp):
                            col = g * ti + t
                            nc.vector.tensor_scalar_mul(
                                tbuf[:, q * ext : (q + 1) * ext],
                                xrep[:, :ext],
                                expnegxi[:, col : col + 1],
                            )
                        nc.scalar.activation(
                            vbuf[:, :gw], tbuf[:, :gw], AF.Ln,
                            bias=1.0, scale=1.0,
                        )
                        for q, t in enumerate(grp):
                            col = g * ti + t
                            idx = done + q
                            for k in range(nch):
                                k0 = k * CH
                                k1 = min(k0 + CH, ext)
                                b = 32 * (k % 3)
                                nc.tensor.matmul(
                                    g_pss[k // 3][b : b + 4, : k1 - k0],
                                    onehot[:, col * NCLS : (col + 1) * NCLS],
                                    vbuf[:, q * ext + k0 : q * ext + k1],
                                    start=(idx == 0),
                                    stop=(idx == len(tiles) - 1),
                                )
                        done += len(grp)
                    for p in range(lv_pieces):
                        bands = min(3, nch - 3 * p)
                        rows = 32 * (bands - 1) + 4
                        gsb = gsp.tile([grows, CH], f32, tag="gs", name="gsb")
                        nc.vector.tensor_copy(gsb[:rows, :], g_pss[p][:rows, :])
                        nc.sync.dma_start(gout_d[g, a - 1, p, :rows, :],
                                          gsb[:rows, :])
    nc.compile()
    return nc


def _prepare_core(logits, labels, scls):
    """Host-side layout prep for one core's GPC graphs."""
    import ml_dtypes
    jstart, lj, jext, istart, li_raw, ti, levels = _layout(scls)
    expxj = np.zeros((GPC, max(lj, 1)), ml_dtypes.bfloat16)
    expnegxi = np.zeros((GPC, P, max(ti, 1)), np.float32)
    onehot = np.zeros((GPC, max(ti, 1), P, NCLS), ml_dtypes.bfloat16)
    for g in range(GPC):
        x = logits[g].astype(np.float64)
        lab = labels[g]
        for c in (0, 1, 2):
            xc = x[lab == c]
            expxj[g, jstart[c] : jstart[c] + xc.size] = np.exp(xc)
        ivals = np.zeros(P * max(ti, 1), np.float64)
        ioh = np.zeros((P * max(ti, 1), NCLS), np.float32)
        for a in (1, 2, 3):
            xa = x[lab == a]
            i0 = istart[a]
            ivals[i0 : i0 + xa.size] = np.exp(-xa)
            ioh[i0 : i0 + xa.size, a] = 1.0
        expnegxi[g] = ivals.reshape(max(ti, 1), P).T.astype(np.float32)
        onehot[g] = ioh.reshape(max(ti, 1), P, NCLS)
    expnegxi_sb = np.ascontiguousarray(
        expnegxi.transpose(1, 0, 2).reshape(P, GPC * max(ti, 1)))
    onehot_sb = np.ascontiguousarray(
        onehot.transpose(2, 0, 1, 3).reshape(P, GPC * max(ti, 1) * NCLS))
    return {"expxj": expxj, "expnegxi": expnegxi_sb, "onehot": onehot_sb}


def _assemble(g_all, counts, scls):
    """Host-side final math from device G matrices. g_all: [B,3,4*mc,CH]."""
    jstart, lj, jext, istart, li_raw, ti, levels = _layout(scls)
    max_chunks = max(1, -(-lj // CH))
    have_level = {a: any(lv == a for lv in levels) and jext[a] > 0
                  for a in (1, 2, 3)}
    # unpack piece/band layout to [B, 3, 4, lj]
    gm = np.zeros((B, 3, 4, max_chunks * CH), np.float64)
    for k in range(max_chunks):
        p, b = k // 3, 32 * (k % 3)
        gm[:, :, :, k * CH : (k + 1) * CH] = g_all[:, :, p, b : b + 4, :]
    per_graph = np.zeros(B, np.float64)
    for g in range(B):
        n = counts[g]
        means = []
        valids = []
        for lvl in (1, 2, 3):
            s_dev = 0.0
            for a in range(lvl, 4):
                if not have_level.get(a, False):
                    continue
                for c in range(lvl):
                    c0, c1 = jstart[c], jstart[c] + scls[c]
                    if c1 > c0:
                        s_dev += gm[g, a - 1, lvl, c0:c1].sum()
            s_ref = -s_dev
            cnt = float(n[lvl]) * float(n[:lvl].sum())
            valid = cnt > 0
            means.append(s_ref / max(cnt, 1.0) if valid else 0.0)
            valids.append(1.0 if valid else 0.0)
        per_graph[g] = sum(means) / max(sum(valids), 1.0)
    return np.float32(-per_graph.mean())


def kernel(logits, labels):
    logits = np.ascontiguousarray(np.asarray(logits, np.float32))
    labels = np.ascontiguousarray(np.asarray(labels, np.int32))
    assert logits.shape == (B, N) and labels.shape == (B, N)

    counts = np.stack([(labels == c).sum(1) for c in range(NCLS)], axis=1)  # [B,4]
    # even segment sizes keep every extent/chunk slice 4-byte aligned in bf16
    scls = tuple(int(counts[:, c].max() + 1) // 2 * 2 for c in range(NCLS))

    if scls not in _BUILD_CACHE:
        _BUILD_CACHE[scls] = _build(scls)
    nc = _BUILD_CACHE[scls]

    in_maps = [
        _prepare_core(logits[c * GPC : (c + 1) * GPC],
                      labels[c * GPC : (c + 1) * GPC], scls)
        for c in range(N_CORES)
    ]
    res = run_bass_kernel_spmd(nc, in_maps, list(range(N_CORES)))
    g_all = np.concatenate(
        [res.results[c]["gout"] for c in range(N_CORES)], axis=0)
    return _assemble(g_all, counts, scls)


if __name__ == "__main__":
    rng = np.random.default_rng(0)
    lg = rng.normal(size=(B, N)).astype(np.float32)
    lb = rng.integers(0, NCLS, size=(B, N)).astype(np.int32)
    print(kernel(lg, lb))
